# revision 32
# baseline (speedup 1.0000x reference)
"""ADL (attention-dropout-layer) forward kernel for Trainium2, 8 NeuronCores.

Pure data parallel: batch 64 is split 8 ways; each core handles 8 samples.
Per sample: 1x1 conv (channel contraction) -> logits z [1024 spatial],
drop the top-M spatial positions (mask=0), keep bottom (1024-M) (mask=1),
output fm * mask and attn = sigmoid(z + b).

v4: samples processed in 4 groups of 2; each group's feature map stays
resident in SBUF between the logit pass and the mask-multiply (single HBM
read instead of two). The per-sample drop threshold (the M-th largest
logit) is found by radix-33 search: each sample's 1024 logits are
replicated across 32 SBUF partitions, so one fused compare+accumulate DVE
op counts 32 candidate thresholds at once; a constant block-ones matmul
sums the 32 cell indicators per sample and broadcasts the result, and the
search interval shrinks 33x per step - 8 steps reach ~1e-10 of the
initial width. Sigmoid is monotone so logit order == attn order, and the
conv bias shifts all logits equally - neither changes the selection.

Self-contained: hardcodes shapes B,C,H,W = 64,1024,32,32 and n_cores=8.
"""

from contextlib import ExitStack

import numpy as np

import concourse.bacc as bacc
import concourse.bass_isa as bass_isa
import concourse.bass as bass
import concourse.mybir as mybir
from concourse.tile import TileContext
from concourse.bass_utils import run_bass_kernel_spmd

N_CORES = 8
B, C, H, W = 64, 1024, 32, 32
HW = H * W            # 1024 spatial positions
BS = B // N_CORES     # 8 samples per core
KC = C // 128         # 8 channel chunks of 128
GS = 2                # max samples per resident group
GROUP_SIZES = (1, 1, 2, 2, 1, 1)
NT = 32               # thresholds evaluated per radix step (replicas/sample)
SP = GS * NT          # selection partitions per max-size group (64)
RADIX_ITERS = 7       # interval shrinks 33x per iter
F32 = mybir.dt.float32
I32 = mybir.dt.int32


def build_nc(m_drop: int, bias_val: float) -> bass.Bass:
    nc = bacc.Bacc(None, target_bir_lowering=False)

    fm = nc.declare_dram_parameter("feature_maps", [BS, C, H, W], F32, isOutput=False)
    w = nc.declare_dram_parameter("conv_w", [C], F32, isOutput=False)
    dropped = nc.declare_dram_parameter("dropped", [BS, C, H, W], F32, isOutput=True)
    attn = nc.declare_dram_parameter("attn", [BS, 1, H, W], F32, isOutput=True)

    fm_ap = fm[:].rearrange("b c h w -> b c (h w)")          # [8, 1024, 1024]
    drop_ap = dropped[:].rearrange("b c h w -> b c (h w)")   # [8, 1024, 1024]
    attn_ap = attn[:].rearrange("b o h w -> b (o h w)")      # [8, 1024]

    with TileContext(nc) as tc, ExitStack() as ctx:
        singles = ctx.enter_context(tc.tile_pool(name="singles", bufs=1))
        res = ctx.enter_context(tc.tile_pool(name="res", bufs=2 * GS * KC + 4))
        fm_out = ctx.enter_context(tc.tile_pool(name="fm_out", bufs=3))
        mask_sbp = ctx.enter_context(tc.tile_pool(name="mask_sb", bufs=2))
        zrow_p = ctx.enter_context(tc.tile_pool(name="zrow", bufs=1))
        small = ctx.enter_context(tc.tile_pool(name="small", bufs=2))
        psum_z = ctx.enter_context(tc.tile_pool(name="psum_z", bufs=1, space="PSUM"))
        psum_m = ctx.enter_context(tc.tile_pool(name="psum_m", bufs=1, space="PSUM"))
        psum_s = ctx.enter_context(tc.tile_pool(name="psum_s", bufs=1, space="PSUM"))
        psum_c = ctx.enter_context(tc.tile_pool(name="psum_c", bufs=2, space="PSUM"))

        # conv weight, chunked: w_sb[p, k] = w[k*128 + p]
        w_sb = singles.tile([128, KC], F32)
        nc.sync.dma_start(out=w_sb, in_=w[:].rearrange("(k p) -> p k", p=128))

        # one-hot lhsT for broadcasting mask row j (of GS) to 128 partitions:
        # oh[k, j*128 + i] = 1 iff k == j
        ones_g = singles.tile([GS, GS * 128], F32)
        nc.vector.memset(ones_g, 1.0)
        oh = singles.tile([GS, GS * 128], F32)
        nc.gpsimd.affine_select(
            out=oh, in_=ones_g,
            pattern=[[-1, GS], [0, 128]],
            compare_op=mybir.AluOpType.is_equal,
            fill=0.0, base=0, channel_multiplier=1,
        )

        # block-ones matmul weight: blk[k, m] = 1 iff k//NT == m//NT.
        # out = blk.T @ x sums each sample's NT partitions and broadcasts
        # the sum back to all NT partitions of that sample.
        ones_s = singles.tile([SP, SP], F32)
        nc.vector.memset(ones_s, 1.0)
        blk_lo = singles.tile([SP, SP], F32)
        nc.gpsimd.affine_select(
            out=blk_lo, in_=ones_s,
            pattern=[[-NT, GS], [0, NT]],
            compare_op=mybir.AluOpType.is_ge,
            fill=0.0, base=0, channel_multiplier=1,
        )
        blk = singles.tile([SP, SP], F32)
        nc.gpsimd.affine_select(
            out=blk, in_=blk_lo,
            pattern=[[NT, GS], [0, NT]],
            compare_op=mybir.AluOpType.is_ge,
            fill=0.0, base=NT - 1, channel_multiplier=-1,
        )

        # per-partition threshold index: rconst[q] = (q % NT) + 1
        qidx = singles.tile([SP, 1], I32)
        nc.gpsimd.iota(qidx, pattern=[[0, 1]], base=0, channel_multiplier=1)
        nc.vector.tensor_scalar(
            out=qidx, in0=qidx, scalar1=NT - 1, scalar2=None,
            op0=mybir.AluOpType.bitwise_and,
        )
        rconst = singles.tile([SP, 1], F32)
        nc.vector.tensor_copy(out=rconst, in_=qidx)
        nc.vector.tensor_scalar_add(rconst, rconst, 1.0)

        # pow33[:, i] = 33^-(i+1): cell width fraction after each step
        pow33 = singles.tile([SP, RADIX_ITERS + 1], F32)
        for i in range(RADIX_ITERS + 1):
            nc.vector.memset(pow33[:, i : i + 1], float((NT + 1.0) ** (-(i + 1))))

        s0 = 0
        for gs in GROUP_SIZES:
            sp = gs * NT
            # ---- load group + logits via PE
            tiles = []
            zrep_full = small.tile([SP, HW], F32, tag="zrep", name="zrep")
            zrep = zrep_full[:sp, :]
            for j in range(gs):
                s = s0 + j
                zp = psum_z.tile([1, HW], F32)
                for k in range(KC):
                    t = res.tile([128, HW], F32, tag="res")
                    deng = nc.sync if k % 2 == 0 else nc.scalar
                    deng.dma_start(out=t, in_=fm_ap[s, k * 128 : (k + 1) * 128, :])
                    tiles.append(t)
                    for n in range(2):
                        nc.tensor.matmul(
                            zp[0:1, n * 512 : (n + 1) * 512],
                            lhsT=w_sb[:, k : k + 1],
                            rhs=t[:, n * 512 : (n + 1) * 512],
                            start=(k == 0),
                            stop=(k == KC - 1),
                        )
                zr = zrow_p.tile([1, HW], F32)
                nc.scalar.copy(out=zr, in_=zp[0:1, :])
                # replicate sample j's logits across its NT partitions
                # (source repeats via a zero-stride free axis)
                nc.sync.dma_start(
                    out=zrep[j * NT : (j + 1) * NT, :],
                    in_=zr.unsqueeze(1).to_broadcast([1, NT, HW]),
                )
                # attn = sigmoid(z + b), from the psum row copy (in place)
                nc.scalar.activation(
                    out=zr, in_=zr,
                    func=mybir.ActivationFunctionType.Sigmoid,
                    bias=bias_val, scale=1.0,
                )
                nc.sync.dma_start(out=attn_ap[s : s + 1, :], in_=zr)

            mask2_full = small.tile([GS, HW], F32, tag="mask2", name="mask2")
            mask2 = mask2_full[:gs, :]
            if m_drop <= 0:
                nc.vector.memset(mask2, 1.0)
            else:
                # ---- bounds: every selection partition holds the full sample
                lo = small.tile([SP, 1], F32, tag="lo", name="lo")[:sp, :]
                hi0 = small.tile([SP, 1], F32, tag="hi0", name="hi0")[:sp, :]
                nc.vector.tensor_reduce(
                    out=lo, in_=zrep, axis=mybir.AxisListType.X,
                    op=mybir.AluOpType.min,
                )
                nc.vector.tensor_scalar_add(lo, lo, -1.0)
                nc.vector.tensor_reduce(
                    out=hi0, in_=zrep, axis=mybir.AxisListType.X,
                    op=mybir.AluOpType.max,
                )
                w0 = small.tile([SP, 1], F32, tag="w0", name="w0")[:sp, :]
                nc.vector.scalar_tensor_tensor(
                    out=w0, in0=hi0, scalar=2.0, in1=lo,
                    op0=mybir.AluOpType.add, op1=mybir.AluOpType.subtract,
                )
                # widths table: wh[:, i] = w0 * 33^-(i+1)
                wh = small.tile([SP, RADIX_ITERS + 1], F32, tag="wh", name="wh")[:sp, :]
                nc.vector.tensor_scalar(
                    out=wh, in0=pow33[:sp, :], scalar1=w0, scalar2=None,
                    op0=mybir.AluOpType.mult,
                )
                # initial thresholds: thr[q] = lo + rconst[q] * wh[:, 0]
                thr = small.tile([SP, 1], F32, tag="thr", name="thr")[:sp, :]
                nc.vector.scalar_tensor_tensor(
                    out=thr, in0=rconst[:sp, :], scalar=wh[:, 0:1], in1=lo,
                    op0=mybir.AluOpType.mult, op1=mybir.AluOpType.add,
                )

                cnt = small.tile([SP, 1], F32, tag="cnt", name="cnt")[:sp, :]
                ind = small.tile([SP, 1], F32, tag="ind", name="ind")[:sp, :]
                for i in range(RADIX_ITERS):
                    scr = psum_s.tile([SP, HW], F32, tag="scr", name="scr")[:sp, :]
                    # cnt[q] = #{f: z[f] >= thr[q]} (full sample per partition)
                    nc.vector.tensor_scalar(
                        out=scr, in0=zrep, scalar1=thr, scalar2=None,
                        op0=mybir.AluOpType.is_ge, op1=mybir.AluOpType.add,
                        accum_out=cnt,
                    )
                    nc.vector.tensor_scalar(
                        out=ind, in0=cnt, scalar1=float(m_drop), scalar2=None,
                        op0=mybir.AluOpType.is_ge,
                    )
                    # selc = #thresholds of this sample with count >= M,
                    # broadcast to all its partitions
                    selc = psum_c.tile([SP, 1], F32, tag="selc", name="selc")[:sp, :]
                    nc.tensor.matmul(
                        selc, lhsT=blk[:sp, :sp], rhs=ind, start=True, stop=True
                    )
                    # lo += selc * wh[:, i]
                    nc.vector.scalar_tensor_tensor(
                        out=lo, in0=selc, scalar=wh[:, i : i + 1], in1=lo,
                        op0=mybir.AluOpType.mult, op1=mybir.AluOpType.add,
                    )
                    if i + 1 < RADIX_ITERS:
                        nc.vector.scalar_tensor_tensor(
                            out=thr, in0=rconst[:sp, :], scalar=wh[:, i + 1 : i + 2], in1=lo,
                            op0=mybir.AluOpType.mult, op1=mybir.AluOpType.add,
                        )
                # keep-mask: 1 where z < lo (the HW - M smallest logits);
                # computed in place over the replica layout, rows {0, NT}
                # then hold sample 0/1 masks
                nc.vector.tensor_scalar(
                    out=zrep, in0=zrep, scalar1=lo, scalar2=None,
                    op0=mybir.AluOpType.is_lt,
                )
                for j in range(gs):
                    nc.sync.dma_start(
                        out=mask2[j : j + 1, :],
                        in_=zrep[j * NT : j * NT + 1, :],
                    )

            # ---- apply mask to the resident tiles, stream out
            for j in range(gs):
                s = s0 + j
                mp = psum_m.tile([128, HW], F32)
                for n in range(2):
                    nc.tensor.matmul(
                        mp[:, n * 512 : (n + 1) * 512],
                        lhsT=oh[0:gs, j * 128 : (j + 1) * 128],
                        rhs=mask2[:, n * 512 : (n + 1) * 512],
                        start=True,
                        stop=True,
                    )
                mask_sb = mask_sbp.tile([128, HW], F32)
                nc.scalar.copy(out=mask_sb, in_=mp)
                for k in range(KC):
                    o = fm_out.tile([128, HW], F32)
                    nc.vector.tensor_mul(out=o, in0=tiles[j * KC + k], in1=mask_sb)
                    deng = nc.scalar if k % 2 == 0 else nc.sync
                    deng.dma_start(
                        out=drop_ap[s, k * 128 : (k + 1) * 128, :], in_=o
                    )
            s0 += gs

    nc.compile()
    return nc


_CACHE: dict = {}


def _get_nc(m_drop: int, bias_val: float) -> bass.Bass:
    key = (m_drop, bias_val)
    if key not in _CACHE:
        _CACHE[key] = build_nc(m_drop, bias_val)
    return _CACHE[key]


def _run(feature_maps, conv_w, conv_b, M, trace=False):
    fm = np.ascontiguousarray(np.asarray(feature_maps, dtype=np.float32))
    w = np.ascontiguousarray(np.asarray(conv_w, dtype=np.float32))
    b = np.asarray(conv_b, dtype=np.float32)
    m_drop = int(M)
    nc = _get_nc(m_drop, float(b[0]))
    in_maps = [
        {"feature_maps": fm[i * BS : (i + 1) * BS], "conv_w": w}
        for i in range(N_CORES)
    ]
    res = run_bass_kernel_spmd(nc, in_maps, list(range(N_CORES)), trace=trace)
    results = res.results
    dropped = np.concatenate([results[i]["dropped"] for i in range(N_CORES)], axis=0)
    attn = np.concatenate([results[i]["attn"] for i in range(N_CORES)], axis=0)
    return (dropped, attn), res


def kernel(feature_maps, conv_w, conv_b, M):
    (dropped, attn), _ = _run(feature_maps, conv_w, conv_b, M)
    return dropped, attn


# revision 33
# speedup vs baseline: 1.0134x; 1.0134x over previous
"""ADL (attention-dropout-layer) forward kernel for Trainium2, 8 NeuronCores.

Pure data parallel: batch 64 is split 8 ways; each core handles 8 samples.
Per sample: 1x1 conv (channel contraction) -> logits z [1024 spatial],
drop the top-M spatial positions (mask=0), keep bottom (1024-M) (mask=1),
output fm * mask and attn = sigmoid(z + b).

v4: samples processed in 4 groups of 2; each group's feature map stays
resident in SBUF between the logit pass and the mask-multiply (single HBM
read instead of two). The per-sample drop threshold (the M-th largest
logit) is found by radix-33 search: each sample's 1024 logits are
replicated across 32 SBUF partitions, so one fused compare+accumulate DVE
op counts 32 candidate thresholds at once; a constant block-ones matmul
sums the 32 cell indicators per sample and broadcasts the result, and the
search interval shrinks 33x per step - 8 steps reach ~1e-10 of the
initial width. Sigmoid is monotone so logit order == attn order, and the
conv bias shifts all logits equally - neither changes the selection.

Self-contained: hardcodes shapes B,C,H,W = 64,1024,32,32 and n_cores=8.
"""

from contextlib import ExitStack

import numpy as np

import concourse.bacc as bacc
import concourse.bass as bass
import concourse.mybir as mybir
from concourse.tile import TileContext
from concourse.bass_utils import run_bass_kernel_spmd

N_CORES = 8
B, C, H, W = 64, 1024, 32, 32
HW = H * W            # 1024 spatial positions
BS = B // N_CORES     # 8 samples per core
KC = C // 128         # 8 channel chunks of 128
GS = 2                # max samples per resident group
GROUP_SIZES = (1, 1, 2, 2, 1, 1)
NT = 32               # thresholds evaluated per radix step (replicas/sample)
SP = GS * NT          # selection partitions per max-size group (64)
RADIX_ITERS = 7       # interval shrinks 33x per iter
F32 = mybir.dt.float32
I32 = mybir.dt.int32


def build_nc(m_drop: int, bias_val: float) -> bass.Bass:
    nc = bacc.Bacc(None, target_bir_lowering=False)

    fm = nc.declare_dram_parameter("feature_maps", [BS, C, H, W], F32, isOutput=False)
    w = nc.declare_dram_parameter("conv_w", [C], F32, isOutput=False)
    dropped = nc.declare_dram_parameter("dropped", [BS, C, H, W], F32, isOutput=True)
    attn = nc.declare_dram_parameter("attn", [BS, 1, H, W], F32, isOutput=True)

    fm_ap = fm[:].rearrange("b c h w -> b c (h w)")          # [8, 1024, 1024]
    drop_ap = dropped[:].rearrange("b c h w -> b c (h w)")   # [8, 1024, 1024]
    attn_ap = attn[:].rearrange("b o h w -> b (o h w)")      # [8, 1024]

    with TileContext(nc) as tc, ExitStack() as ctx:
        singles = ctx.enter_context(tc.tile_pool(name="singles", bufs=1))
        res = ctx.enter_context(tc.tile_pool(name="res", bufs=2 * GS * KC + 4))
        fm_out = ctx.enter_context(tc.tile_pool(name="fm_out", bufs=3))
        mask_sbp = ctx.enter_context(tc.tile_pool(name="mask_sb", bufs=2))
        zrow_p = ctx.enter_context(tc.tile_pool(name="zrow", bufs=1))
        small = ctx.enter_context(tc.tile_pool(name="small", bufs=2))
        psum_z = ctx.enter_context(tc.tile_pool(name="psum_z", bufs=1, space="PSUM"))
        psum_m = ctx.enter_context(tc.tile_pool(name="psum_m", bufs=1, space="PSUM"))
        psum_s = ctx.enter_context(tc.tile_pool(name="psum_s", bufs=1, space="PSUM"))
        psum_c = ctx.enter_context(tc.tile_pool(name="psum_c", bufs=2, space="PSUM"))

        # conv weight, chunked: w_sb[p, k] = w[k*128 + p]
        w_sb = singles.tile([128, KC], F32)
        nc.sync.dma_start(out=w_sb, in_=w[:].rearrange("(k p) -> p k", p=128))

        # one-hot lhsT for broadcasting mask row j (of GS) to 128 partitions:
        # oh[k, j*128 + i] = 1 iff k == j
        ones_g = singles.tile([GS, GS * 128], F32)
        nc.vector.memset(ones_g, 1.0)
        oh = singles.tile([GS, GS * 128], F32)
        nc.gpsimd.affine_select(
            out=oh, in_=ones_g,
            pattern=[[-1, GS], [0, 128]],
            compare_op=mybir.AluOpType.is_equal,
            fill=0.0, base=0, channel_multiplier=1,
        )

        # block-ones matmul weight: blk[k, m] = 1 iff k//NT == m//NT.
        # out = blk.T @ x sums each sample's NT partitions and broadcasts
        # the sum back to all NT partitions of that sample.
        ones_s = singles.tile([SP, SP], F32)
        nc.vector.memset(ones_s, 1.0)
        blk_lo = singles.tile([SP, SP], F32)
        nc.gpsimd.affine_select(
            out=blk_lo, in_=ones_s,
            pattern=[[-NT, GS], [0, NT]],
            compare_op=mybir.AluOpType.is_ge,
            fill=0.0, base=0, channel_multiplier=1,
        )
        blk = singles.tile([SP, SP], F32)
        nc.gpsimd.affine_select(
            out=blk, in_=blk_lo,
            pattern=[[NT, GS], [0, NT]],
            compare_op=mybir.AluOpType.is_ge,
            fill=0.0, base=NT - 1, channel_multiplier=-1,
        )

        # per-partition threshold index: rconst[q] = (q % NT) + 1
        qidx = singles.tile([SP, 1], I32)
        nc.gpsimd.iota(qidx, pattern=[[0, 1]], base=0, channel_multiplier=1)
        nc.vector.tensor_scalar(
            out=qidx, in0=qidx, scalar1=NT - 1, scalar2=None,
            op0=mybir.AluOpType.bitwise_and,
        )
        rconst = singles.tile([SP, 1], F32)
        nc.vector.tensor_copy(out=rconst, in_=qidx)
        nc.vector.tensor_scalar_add(rconst, rconst, 1.0)

        # pow33[:, i] = 33^-(i+1): cell width fraction after each step
        pow33 = singles.tile([SP, RADIX_ITERS + 1], F32)
        for i in range(RADIX_ITERS + 1):
            nc.vector.memset(pow33[:, i : i + 1], float((NT + 1.0) ** (-(i + 1))))

        s0 = 0
        for gs in GROUP_SIZES:
            sp = gs * NT
            # ---- load group + logits via PE
            tiles = []
            zrep_full = small.tile([SP, HW], F32, tag="zrep", name="zrep")
            zrep = zrep_full[:sp, :]
            for j in range(gs):
                s = s0 + j
                zp = psum_z.tile([1, HW], F32)
                for k in range(KC):
                    t = res.tile([128, HW], F32, tag="res")
                    deng = nc.sync if k % 2 == 0 else nc.scalar
                    deng.dma_start(out=t, in_=fm_ap[s, k * 128 : (k + 1) * 128, :])
                    tiles.append(t)
                    for n in range(2):
                        nc.tensor.matmul(
                            zp[0:1, n * 512 : (n + 1) * 512],
                            lhsT=w_sb[:, k : k + 1],
                            rhs=t[:, n * 512 : (n + 1) * 512],
                            start=(k == 0),
                            stop=(k == KC - 1),
                        )
                zr = zrow_p.tile([1, HW], F32)
                nc.scalar.copy(out=zr, in_=zp[0:1, :])
                # replicate sample j's logits across its NT partitions
                # (source repeats via a zero-stride free axis)
                nc.sync.dma_start(
                    out=zrep[j * NT : (j + 1) * NT, :],
                    in_=zr.unsqueeze(1).to_broadcast([1, NT, HW]),
                )
                # attn = sigmoid(z + b), from the psum row copy (in place)
                nc.scalar.activation(
                    out=zr, in_=zr,
                    func=mybir.ActivationFunctionType.Sigmoid,
                    bias=bias_val, scale=1.0,
                )
                nc.sync.dma_start(out=attn_ap[s : s + 1, :], in_=zr)

            mask2_full = small.tile([GS, HW], F32, tag="mask2", name="mask2")
            mask2 = mask2_full[:gs, :]
            if m_drop <= 0:
                nc.vector.memset(mask2, 1.0)
            else:
                # ---- bounds: every selection partition holds the full sample
                lo = small.tile([SP, 1], F32, tag="lo", name="lo")[:sp, :]
                hi0 = small.tile([SP, 1], F32, tag="hi0", name="hi0")[:sp, :]
                nc.vector.tensor_reduce(
                    out=lo, in_=zrep, axis=mybir.AxisListType.X,
                    op=mybir.AluOpType.min,
                )
                nc.vector.tensor_scalar_add(lo, lo, -1.0)
                nc.vector.tensor_reduce(
                    out=hi0, in_=zrep, axis=mybir.AxisListType.X,
                    op=mybir.AluOpType.max,
                )
                w0 = small.tile([SP, 1], F32, tag="w0", name="w0")[:sp, :]
                nc.vector.scalar_tensor_tensor(
                    out=w0, in0=hi0, scalar=2.0, in1=lo,
                    op0=mybir.AluOpType.add, op1=mybir.AluOpType.subtract,
                )
                # widths table: wh[:, i] = w0 * 33^-(i+1)
                wh = small.tile([SP, RADIX_ITERS + 1], F32, tag="wh", name="wh")[:sp, :]
                nc.vector.tensor_scalar(
                    out=wh, in0=pow33[:sp, :], scalar1=w0, scalar2=None,
                    op0=mybir.AluOpType.mult,
                )
                # initial thresholds: thr[q] = lo + rconst[q] * wh[:, 0]
                thr = small.tile([SP, 1], F32, tag="thr", name="thr")[:sp, :]
                nc.vector.scalar_tensor_tensor(
                    out=thr, in0=rconst[:sp, :], scalar=wh[:, 0:1], in1=lo,
                    op0=mybir.AluOpType.mult, op1=mybir.AluOpType.add,
                )

                cnt = small.tile([SP, 1], F32, tag="cnt", name="cnt")[:sp, :]
                ind = small.tile([SP, 1], F32, tag="ind", name="ind")[:sp, :]
                for i in range(RADIX_ITERS):
                    scr = psum_s.tile([SP, HW], F32, tag="scr", name="scr")[:sp, :]
                    # cnt[q] = #{f: z[f] >= thr[q]} (full sample per partition)
                    nc.vector.tensor_scalar(
                        out=scr, in0=zrep, scalar1=thr, scalar2=None,
                        op0=mybir.AluOpType.is_ge, op1=mybir.AluOpType.add,
                        accum_out=cnt,
                    )
                    nc.vector.tensor_scalar(
                        out=ind, in0=cnt, scalar1=float(m_drop), scalar2=None,
                        op0=mybir.AluOpType.is_ge,
                    )
                    # selc = #thresholds of this sample with count >= M,
                    # broadcast to all its partitions
                    selc = psum_c.tile([SP, 1], F32, tag="selc", name="selc")[:sp, :]
                    nc.tensor.matmul(
                        selc, lhsT=blk[:sp, :sp], rhs=ind, start=True, stop=True
                    )
                    # lo += selc * wh[:, i]
                    nc.vector.scalar_tensor_tensor(
                        out=lo, in0=selc, scalar=wh[:, i : i + 1], in1=lo,
                        op0=mybir.AluOpType.mult, op1=mybir.AluOpType.add,
                    )
                    if i + 1 < RADIX_ITERS:
                        nc.vector.scalar_tensor_tensor(
                            out=thr, in0=rconst[:sp, :], scalar=wh[:, i + 1 : i + 2], in1=lo,
                            op0=mybir.AluOpType.mult, op1=mybir.AluOpType.add,
                        )
                # keep-mask: 1 where z < lo (the HW - M smallest logits);
                # computed in place over the replica layout, rows {0, NT}
                # then hold sample 0/1 masks
                nc.vector.tensor_scalar(
                    out=zrep, in0=zrep, scalar1=lo, scalar2=None,
                    op0=mybir.AluOpType.is_lt,
                )
                for j in range(gs):
                    nc.sync.dma_start(
                        out=mask2[j : j + 1, :],
                        in_=zrep[j * NT : j * NT + 1, :],
                    )

            # ---- apply mask to the resident tiles, stream out
            for j in range(gs):
                s = s0 + j
                mp = psum_m.tile([128, HW], F32)
                for n in range(2):
                    nc.tensor.matmul(
                        mp[:, n * 512 : (n + 1) * 512],
                        lhsT=oh[0:gs, j * 128 : (j + 1) * 128],
                        rhs=mask2[:, n * 512 : (n + 1) * 512],
                        start=True,
                        stop=True,
                    )
                mask_sb = mask_sbp.tile([128, HW], F32)
                nc.scalar.copy(out=mask_sb, in_=mp)
                for k in range(KC):
                    o = fm_out.tile([128, HW], F32)
                    nc.vector.tensor_mul(out=o, in0=tiles[j * KC + k], in1=mask_sb)
                    deng = nc.scalar if k % 2 == 0 else nc.sync
                    deng.dma_start(
                        out=drop_ap[s, k * 128 : (k + 1) * 128, :], in_=o
                    )
            s0 += gs

    nc.compile()
    return nc


_CACHE: dict = {}


def _get_nc(m_drop: int, bias_val: float) -> bass.Bass:
    key = (m_drop, bias_val)
    if key not in _CACHE:
        _CACHE[key] = build_nc(m_drop, bias_val)
    return _CACHE[key]


def _run(feature_maps, conv_w, conv_b, M, trace=False):
    fm = np.ascontiguousarray(np.asarray(feature_maps, dtype=np.float32))
    w = np.ascontiguousarray(np.asarray(conv_w, dtype=np.float32))
    b = np.asarray(conv_b, dtype=np.float32)
    m_drop = int(M)
    nc = _get_nc(m_drop, float(b[0]))
    in_maps = [
        {"feature_maps": fm[i * BS : (i + 1) * BS], "conv_w": w}
        for i in range(N_CORES)
    ]
    res = run_bass_kernel_spmd(nc, in_maps, list(range(N_CORES)), trace=trace)
    results = res.results
    dropped = np.concatenate([results[i]["dropped"] for i in range(N_CORES)], axis=0)
    attn = np.concatenate([results[i]["attn"] for i in range(N_CORES)], axis=0)
    return (dropped, attn), res


def kernel(feature_maps, conv_w, conv_b, M):
    (dropped, attn), _ = _run(feature_maps, conv_w, conv_b, M)
    return dropped, attn


# revision 34
# speedup vs baseline: 1.2049x; 1.1889x over previous
"""ADL (attention-dropout-layer) forward kernel for Trainium2, 8 NeuronCores.

Pure data parallel: batch 64 is split 8 ways; each core handles 8 samples.
Per sample: 1x1 conv (channel contraction) -> logits z [1024 spatial],
drop the top-M spatial positions (mask=0), keep bottom (1024-M) (mask=1),
output fm * mask and attn = sigmoid(z + b).

v4: samples processed in 4 groups of 2; each group's feature map stays
resident in SBUF between the logit pass and the mask-multiply (single HBM
read instead of two). The per-sample drop threshold (the M-th largest
logit) is found by radix-33 search: each sample's 1024 logits are
replicated across 32 SBUF partitions, so one fused compare+accumulate DVE
op counts 32 candidate thresholds at once; a constant block-ones matmul
sums the 32 cell indicators per sample and broadcasts the result, and the
search interval shrinks 33x per step - 8 steps reach ~1e-10 of the
initial width. Sigmoid is monotone so logit order == attn order, and the
conv bias shifts all logits equally - neither changes the selection.

Self-contained: hardcodes shapes B,C,H,W = 64,1024,32,32 and n_cores=8.
"""

from contextlib import ExitStack

import numpy as np

import concourse.bacc as bacc
import concourse.bass as bass
import concourse.mybir as mybir
from concourse.tile import TileContext
from concourse.bass_utils import run_bass_kernel_spmd

N_CORES = 8
B, C, H, W = 64, 1024, 32, 32
HW = H * W            # 1024 spatial positions
BS = B // N_CORES     # 8 samples per core
KC = C // 128         # 8 channel chunks of 128
GS = 2                # max samples per resident group
GROUP_SIZES = (1, 1, 2, 2, 1, 1)
NT = 32               # thresholds evaluated per radix step (replicas/sample)
SP = GS * NT          # selection partitions per max-size group (64)
RADIX_ITERS = 7       # interval shrinks 33x per iter
F32 = mybir.dt.float32
I32 = mybir.dt.int32


def build_nc(m_drop: int, bias_val: float) -> bass.Bass:
    nc = bacc.Bacc(None, target_bir_lowering=False)

    fm = nc.declare_dram_parameter("feature_maps", [BS, C, H, W], F32, isOutput=False)
    w = nc.declare_dram_parameter("conv_w", [C], F32, isOutput=False)
    dropped = nc.declare_dram_parameter("dropped", [BS, C, H, W], F32, isOutput=True)
    attn = nc.declare_dram_parameter("attn", [BS, 1, H, W], F32, isOutput=True)

    fm_ap = fm[:].rearrange("b c h w -> b c (h w)")          # [8, 1024, 1024]
    drop_ap = dropped[:].rearrange("b c h w -> b c (h w)")   # [8, 1024, 1024]
    attn_ap = attn[:].rearrange("b o h w -> b (o h w)")      # [8, 1024]

    with TileContext(nc) as tc, ExitStack() as ctx:
        singles = ctx.enter_context(tc.tile_pool(name="singles", bufs=1))
        res = ctx.enter_context(tc.tile_pool(name="res", bufs=2 * GS * KC + 3))
        fm_out = ctx.enter_context(tc.tile_pool(name="fm_out", bufs=3))
        mask_sbp = ctx.enter_context(tc.tile_pool(name="mask_sb", bufs=1))
        zrow_p = ctx.enter_context(tc.tile_pool(name="zrow", bufs=1))
        small = ctx.enter_context(tc.tile_pool(name="small", bufs=2))
        psum_z = ctx.enter_context(tc.tile_pool(name="psum_z", bufs=1, space="PSUM"))
        psum_m = ctx.enter_context(tc.tile_pool(name="psum_m", bufs=1, space="PSUM"))
        psum_s = ctx.enter_context(tc.tile_pool(name="psum_s", bufs=1, space="PSUM"))
        psum_c = ctx.enter_context(tc.tile_pool(name="psum_c", bufs=2, space="PSUM"))

        # conv weight, chunked: w_sb[p, k] = w[k*128 + p]
        w_sb = singles.tile([128, KC], F32)
        nc.sync.dma_start(out=w_sb, in_=w[:].rearrange("(k p) -> p k", p=128))

        # one-hot lhsT for broadcasting mask row j (of GS) to 128 partitions:
        # oh[k, j*128 + i] = 1 iff k == j
        ones_g = singles.tile([GS, GS * 128], F32)
        nc.vector.memset(ones_g, 1.0)
        oh = singles.tile([GS, GS * 128], F32)
        nc.gpsimd.affine_select(
            out=oh, in_=ones_g,
            pattern=[[-1, GS], [0, 128]],
            compare_op=mybir.AluOpType.is_equal,
            fill=0.0, base=0, channel_multiplier=1,
        )

        # block-ones matmul weight: blk[k, m] = 1 iff k//NT == m//NT.
        # out = blk.T @ x sums each sample's NT partitions and broadcasts
        # the sum back to all NT partitions of that sample.
        ones_s = singles.tile([SP, SP], F32)
        nc.vector.memset(ones_s, 1.0)
        blk_lo = singles.tile([SP, SP], F32)
        nc.gpsimd.affine_select(
            out=blk_lo, in_=ones_s,
            pattern=[[-NT, GS], [0, NT]],
            compare_op=mybir.AluOpType.is_ge,
            fill=0.0, base=0, channel_multiplier=1,
        )
        blk = singles.tile([SP, SP], F32)
        nc.gpsimd.affine_select(
            out=blk, in_=blk_lo,
            pattern=[[NT, GS], [0, NT]],
            compare_op=mybir.AluOpType.is_ge,
            fill=0.0, base=NT - 1, channel_multiplier=-1,
        )

        # per-partition threshold index: rconst[q] = (q % NT) + 1
        qidx = singles.tile([SP, 1], I32)
        nc.gpsimd.iota(qidx, pattern=[[0, 1]], base=0, channel_multiplier=1)
        nc.vector.tensor_scalar(
            out=qidx, in0=qidx, scalar1=NT - 1, scalar2=None,
            op0=mybir.AluOpType.bitwise_and,
        )
        rconst = singles.tile([SP, 1], F32)
        nc.vector.tensor_copy(out=rconst, in_=qidx)
        nc.vector.tensor_scalar_add(rconst, rconst, 1.0)

        # pow33[:, i] = 33^-(i+1): cell width fraction after each step
        pow33 = singles.tile([SP, RADIX_ITERS + 1], F32)
        for i in range(RADIX_ITERS + 1):
            nc.vector.memset(pow33[:, i : i + 1], float((NT + 1.0) ** (-(i + 1))))

        s0 = 0
        for gs in GROUP_SIZES:
            sp = gs * NT
            # ---- load group + logits via PE
            tiles = []
            zrep_full = small.tile([SP, HW], F32, tag="zrep", name="zrep")
            zrep = zrep_full[:sp, :]
            for j in range(gs):
                s = s0 + j
                zp = psum_z.tile([1, HW], F32)
                for k in range(KC):
                    t = res.tile([128, HW], F32, tag="res")
                    deng = nc.sync if k % 2 == 0 else nc.scalar
                    deng.dma_start(out=t, in_=fm_ap[s, k * 128 : (k + 1) * 128, :])
                    tiles.append(t)
                    for n in range(2):
                        nc.tensor.matmul(
                            zp[0:1, n * 512 : (n + 1) * 512],
                            lhsT=w_sb[:, k : k + 1],
                            rhs=t[:, n * 512 : (n + 1) * 512],
                            start=(k == 0),
                            stop=(k == KC - 1),
                        )
                zr = zrow_p.tile([1, HW], F32)
                nc.scalar.copy(out=zr, in_=zp[0:1, :])
                # replicate sample j's logits across its NT partitions
                # (source repeats via a zero-stride free axis)
                nc.sync.dma_start(
                    out=zrep[j * NT : (j + 1) * NT, :],
                    in_=zr.unsqueeze(1).to_broadcast([1, NT, HW]),
                )
                # attn = sigmoid(z + b), from the psum row copy (in place)
                nc.scalar.activation(
                    out=zr, in_=zr,
                    func=mybir.ActivationFunctionType.Sigmoid,
                    bias=bias_val, scale=1.0,
                )
                nc.sync.dma_start(out=attn_ap[s : s + 1, :], in_=zr)

            mask2_full = small.tile([GS, HW], F32, tag="mask2", name="mask2")
            mask2 = mask2_full[:gs, :]
            if m_drop <= 0:
                nc.vector.memset(mask2, 1.0)
            else:
                # ---- bounds: every selection partition holds the full sample
                lo = small.tile([SP, 1], F32, tag="lo", name="lo")[:sp, :]
                hi0 = small.tile([SP, 1], F32, tag="hi0", name="hi0")[:sp, :]
                nc.vector.tensor_reduce(
                    out=lo, in_=zrep, axis=mybir.AxisListType.X,
                    op=mybir.AluOpType.min,
                )
                nc.vector.tensor_scalar_add(lo, lo, -1.0)
                nc.vector.tensor_reduce(
                    out=hi0, in_=zrep, axis=mybir.AxisListType.X,
                    op=mybir.AluOpType.max,
                )
                w0 = small.tile([SP, 1], F32, tag="w0", name="w0")[:sp, :]
                nc.vector.scalar_tensor_tensor(
                    out=w0, in0=hi0, scalar=2.0, in1=lo,
                    op0=mybir.AluOpType.add, op1=mybir.AluOpType.subtract,
                )
                # widths table: wh[:, i] = w0 * 33^-(i+1)
                wh = small.tile([SP, RADIX_ITERS + 1], F32, tag="wh", name="wh")[:sp, :]
                nc.vector.tensor_scalar(
                    out=wh, in0=pow33[:sp, :], scalar1=w0, scalar2=None,
                    op0=mybir.AluOpType.mult,
                )
                # initial thresholds: thr[q] = lo + rconst[q] * wh[:, 0]
                thr = small.tile([SP, 1], F32, tag="thr", name="thr")[:sp, :]
                nc.vector.scalar_tensor_tensor(
                    out=thr, in0=rconst[:sp, :], scalar=wh[:, 0:1], in1=lo,
                    op0=mybir.AluOpType.mult, op1=mybir.AluOpType.add,
                )

                cnt = small.tile([SP, 1], F32, tag="cnt", name="cnt")[:sp, :]
                ind = small.tile([SP, 1], F32, tag="ind", name="ind")[:sp, :]
                for i in range(RADIX_ITERS):
                    scr = small.tile([SP, HW], F32, tag="scr", name="scr")[:sp, :]
                    # cnt[q] = #{f: z[f] >= thr[q]} (full sample per partition)
                    nc.vector.tensor_scalar(
                        out=scr, in0=zrep, scalar1=thr, scalar2=None,
                        op0=mybir.AluOpType.is_ge, op1=mybir.AluOpType.add,
                        accum_out=cnt,
                    )
                    nc.vector.tensor_scalar(
                        out=ind, in0=cnt, scalar1=float(m_drop), scalar2=None,
                        op0=mybir.AluOpType.is_ge,
                    )
                    # selc = #thresholds of this sample with count >= M,
                    # broadcast to all its partitions
                    selc = psum_c.tile([SP, 1], F32, tag="selc", name="selc")[:sp, :]
                    nc.tensor.matmul(
                        selc, lhsT=blk[:sp, :sp], rhs=ind, start=True, stop=True
                    )
                    # lo += selc * wh[:, i]
                    nc.vector.scalar_tensor_tensor(
                        out=lo, in0=selc, scalar=wh[:, i : i + 1], in1=lo,
                        op0=mybir.AluOpType.mult, op1=mybir.AluOpType.add,
                    )
                    if i + 1 < RADIX_ITERS:
                        nc.vector.scalar_tensor_tensor(
                            out=thr, in0=rconst[:sp, :], scalar=wh[:, i + 1 : i + 2], in1=lo,
                            op0=mybir.AluOpType.mult, op1=mybir.AluOpType.add,
                        )
                # keep-mask: 1 where z < lo (the HW - M smallest logits);
                # computed in place over the replica layout, rows {0, NT}
                # then hold sample 0/1 masks
                nc.vector.tensor_scalar(
                    out=zrep, in0=zrep, scalar1=lo, scalar2=None,
                    op0=mybir.AluOpType.is_lt,
                )
                for j in range(gs):
                    nc.sync.dma_start(
                        out=mask2[j : j + 1, :],
                        in_=zrep[j * NT : j * NT + 1, :],
                    )

            # ---- apply mask to the resident tiles, stream out
            for j in range(gs):
                s = s0 + j
                mp = psum_m.tile([128, HW], F32)
                for n in range(2):
                    nc.tensor.matmul(
                        mp[:, n * 512 : (n + 1) * 512],
                        lhsT=oh[0:gs, j * 128 : (j + 1) * 128],
                        rhs=mask2[:, n * 512 : (n + 1) * 512],
                        start=True,
                        stop=True,
                    )
                mask_sb = mask_sbp.tile([128, HW], F32)
                nc.scalar.copy(out=mask_sb, in_=mp)
                for k in range(KC):
                    o = fm_out.tile([128, HW], F32)
                    nc.vector.tensor_mul(out=o, in0=tiles[j * KC + k], in1=mask_sb)
                    deng = nc.scalar if k % 2 == 0 else nc.sync
                    deng.dma_start(
                        out=drop_ap[s, k * 128 : (k + 1) * 128, :], in_=o
                    )
            s0 += gs

    nc.compile()
    return nc


_CACHE: dict = {}


def _get_nc(m_drop: int, bias_val: float) -> bass.Bass:
    key = (m_drop, bias_val)
    if key not in _CACHE:
        _CACHE[key] = build_nc(m_drop, bias_val)
    return _CACHE[key]


def _run(feature_maps, conv_w, conv_b, M, trace=False):
    fm = np.ascontiguousarray(np.asarray(feature_maps, dtype=np.float32))
    w = np.ascontiguousarray(np.asarray(conv_w, dtype=np.float32))
    b = np.asarray(conv_b, dtype=np.float32)
    m_drop = int(M)
    nc = _get_nc(m_drop, float(b[0]))
    in_maps = [
        {"feature_maps": fm[i * BS : (i + 1) * BS], "conv_w": w}
        for i in range(N_CORES)
    ]
    res = run_bass_kernel_spmd(nc, in_maps, list(range(N_CORES)), trace=trace)
    results = res.results
    dropped = np.concatenate([results[i]["dropped"] for i in range(N_CORES)], axis=0)
    attn = np.concatenate([results[i]["attn"] for i in range(N_CORES)], axis=0)
    return (dropped, attn), res


def kernel(feature_maps, conv_w, conv_b, M):
    (dropped, attn), _ = _run(feature_maps, conv_w, conv_b, M)
    return dropped, attn
